# revision 12
# baseline (speedup 1.0000x reference)
"""Trainium2 Bass kernel for nn_Attention_9199819948120.

Multi-head causal attention with GPT-NeoX rotary embeddings.
  B=2, S=2048, d_model=2048, 16 heads x d_head=128, rotary_dim=128.

Sharding (8 cores): core c handles batch c//4 and heads [4*(c%4), 4*(c%4)+4).
Each core computes q/k/v projections for its 4 heads, rotary, causal
attention, and a partial W_O projection [S, d_model]. The host sums the 4
partial outputs per batch (the "all-reduce") and adds b_O.

On-core layouts / algorithm:
  - host pre-transposes inputs to xT [d_model, S] and pre-rounds all matmul
    operands to the fp32r grid (e8m11): fp32r matmuls run 4x faster than
    fp32 at near-fp32 accuracy when inputs are pre-rounded.
  - qT/kT [d_head, t] via chunked matmuls (W chunk stationary), rotary
    applied with DVE tensor ops straight out of PSUM.
  - scores computed transposed (scT [k, q]) so the softmax denominator is a
    ones-vector matmul on the PE and exp goes PSUM->SBUF on the scalar
    engine. Scores are N(0,1)-scaled, so no max-subtraction is needed
    (exp cannot overflow; softmax is shift-invariant).
  - v tiles transposed to [k, e] via PE transpose; PV matmul accumulates
    zT [e, q]; normalization by broadcast reciprocal; W_O matmul emits the
    natural [t, d] partial output.
"""

import numpy as np

B = 2
S = 2048
DM = 2048
NH = 16
E = 128
H_PER = 4          # heads per core
N_CORES = 8
NCHUNK = DM // 128  # 16 d_model chunks
NQT = S // 512      # 4 q tiles of 512
ATTN_SCALE = float(np.sqrt(E))
ROTARY_BASE = 10000.0

_CACHE = {}


def _round_fp32r(x: np.ndarray) -> np.ndarray:
    """Round-to-nearest-even onto the fp32r (e8m11) grid."""
    v = np.ascontiguousarray(x, dtype=np.float32).view(np.uint32)
    lsb = (v >> 12) & np.uint32(1)
    v = (v + np.uint32(0x7FF) + lsb) & np.uint32(0xFFFFF000)
    return v.view(np.float32)


def _build_nc():
    import concourse.bacc as bacc
    import concourse.mybir as mybir
    import concourse.tile as tile

    DT = mybir.dt
    AF = mybir.ActivationFunctionType
    f32 = DT.float32
    f32r = DT.float32r

    nc = bacc.Bacc(trn_type="TRN2", target_bir_lowering=False, debug=False)

    xTq_d = nc.dram_tensor("xTq", [DM, S], f32r, kind="ExternalInput")
    xTk_d = nc.dram_tensor("xTk", [DM, S], f32r, kind="ExternalInput")
    xTv_d = nc.dram_tensor("xTv", [DM, S], f32r, kind="ExternalInput")
    wq_d = nc.dram_tensor("wq", [H_PER, DM, E], f32r, kind="ExternalInput")
    wk_d = nc.dram_tensor("wk", [H_PER, DM, E], f32r, kind="ExternalInput")
    wv_d = nc.dram_tensor("wv", [H_PER, DM, E], f32r, kind="ExternalInput")
    wo_d = nc.dram_tensor("wo", [H_PER, E, DM], f32r, kind="ExternalInput")
    cos_d = nc.dram_tensor("cosT", [E, S], f32, kind="ExternalInput")
    sin_d = nc.dram_tensor("sinTs", [E, S], f32, kind="ExternalInput")
    triu_d = nc.dram_tensor("triu", [128, 128], f32, kind="ExternalInput")
    ones2_d = nc.dram_tensor("ones2", [128, 2], f32r, kind="ExternalInput")
    ones1_d = nc.dram_tensor("ones1", [1, 128], f32r, kind="ExternalInput")
    ident_d = nc.dram_tensor("ident", [128, 128], f32r, kind="ExternalInput")
    out_d = nc.dram_tensor("out", [S, DM], f32, kind="ExternalOutput")

    with tile.TileContext(nc) as tc:
        with (
            tc.tile_pool(name="consts", bufs=1) as consts,
            tc.tile_pool(name="persist", bufs=1) as persist,
            tc.tile_pool(name="wsb", bufs=4) as wsbp,
            tc.tile_pool(name="xchunk", bufs=6) as xchunk,
            tc.tile_pool(name="cs", bufs=2) as csp,
            tc.tile_pool(name="rottmp", bufs=2) as rotp,
            tc.tile_pool(name="vtst", bufs=2) as vtstp,
            tc.tile_pool(name="qtt", bufs=5) as qttp,
            tc.tile_pool(name="expp", bufs=5) as expp,
            tc.tile_pool(name="ztn", bufs=4) as ztnp,
            tc.tile_pool(name="outst", bufs=2) as outstp,
            tc.tile_pool(name="smalls", bufs=2) as smalls,
            tc.tile_pool(name="rbp", bufs=2) as rbp,
            tc.tile_pool(name="wop", bufs=8) as wop,
            tc.tile_pool(name="ps", bufs=8, space="PSUM") as ps,
        ):
            triu_sb = consts.tile([128, 128], f32, tag="triu")
            ones2_sb = consts.tile([128, 2], f32r, tag="ones2")
            ones1_sb = consts.tile([1, 128], f32r, tag="ones1")
            ident_sb = consts.tile([128, 128], f32r, tag="ident")
            nc.gpsimd.dma_start(out=triu_sb, in_=triu_d.ap())
            nc.gpsimd.dma_start(out=ones2_sb, in_=ones2_d.ap())
            nc.gpsimd.dma_start(out=ones1_sb, in_=ones1_d.ap())
            nc.gpsimd.dma_start(out=ident_sb, in_=ident_d.ap())

            def load_wo(h, dd, j):
                t = wop.tile([E, 512], f32r, tag="wo", name=f"wo_{j}_{dd}_{h}")
                nc.gpsimd.dma_start(out=t, in_=wo_d.ap()[h][:, dd * 512:(dd + 1) * 512])
                return t

            kT = [persist.tile([E, S], f32r, tag=f"kT{h}", name=f"kT{h}")
                  for h in range(H_PER)]
            v_sb = [persist.tile([128, S], f32r, tag=f"v{h}", name=f"v{h}")
                    for h in range(H_PER)]

            def load_w(w_d, tag):
                tiles = []
                for h in range(H_PER):
                    t = wsbp.tile([128, NCHUNK * E], f32r, tag="wsb",
                                  name=f"w_{tag}{h}")
                    # w[h] is [DM, E] = [(c p), e] -> SBUF [p, (c e)]
                    nc.gpsimd.dma_start(
                        out=t.rearrange("p (c e) -> p c e", e=E),
                        in_=w_d.ap()[h].rearrange("(c p) e -> p c e", p=128))
                    tiles.append(t)
                return tiles

            def proj_tile(x_d, w_tiles, tq, name):
                """Project one 512-token tile for all heads -> 4 psum banks."""
                banks = [ps.tile([128, 512], f32, tag="bank",
                                 name=f"pj_{name}_{tq}_{h}") for h in range(H_PER)]
                for c in range(NCHUNK):
                    xt = xchunk.tile([128, 512], f32r, tag="xc",
                                     name=f"x_{name}_{tq}_{c}")
                    eng = nc.sync if c % 2 == 0 else nc.scalar
                    eng.dma_start(
                        out=xt,
                        in_=x_d.ap()[c * 128:(c + 1) * 128, tq * 512:(tq + 1) * 512])
                    for h in range(H_PER):
                        nc.tensor.matmul(
                            out=banks[h],
                            lhsT=w_tiles[h][:, c * E:(c + 1) * E],
                            rhs=xt, start=(c == 0), stop=(c == NCHUNK - 1)
                            ).annotate(f"projMM_{name}")
                return banks

            def load_cs(tq, name):
                c_t = csp.tile([E, 512], f32, tag="cos", name=f"cos_{name}_{tq}")
                s_t = csp.tile([E, 512], f32, tag="sin", name=f"sin_{name}_{tq}")
                nc.gpsimd.dma_start(out=c_t, in_=cos_d.ap()[:, tq * 512:(tq + 1) * 512])
                nc.gpsimd.dma_start(out=s_t, in_=sin_d.ap()[:, tq * 512:(tq + 1) * 512])
                return c_t, s_t

            def rotary(bank, c_t, s_t, dst, name):
                """dst = bank * cos + swap_halves(bank) * sin_signed."""
                t1 = rotp.tile([128, 512], f32, tag="r1", name=f"rc_{name}")
                t2 = rotp.tile([128, 512], f32, tag="r2", name=f"rs_{name}")
                nc.vector.tensor_mul(t1, bank, c_t)
                nc.vector.tensor_mul(t2[0:64, :], bank[64:128, :], s_t[0:64, :])
                nc.vector.tensor_mul(t2[64:128, :], bank[0:64, :], s_t[64:128, :])
                nc.vector.tensor_add(dst, t1, t2)

            # ---------------- K phase (rotary deferred one tile) ----------------
            wk_sb = load_w(wk_d, "k")
            pend_k = {}
            for tq in range(NQT + 1):
                if tq < NQT:
                    pend_k[tq] = (proj_tile(xTk_d, wk_sb, tq, "k"),
                                  load_cs(tq, "k"))
                if tq - 1 in pend_k:
                    banks, (c_t, s_t) = pend_k.pop(tq - 1)
                    for h in range(H_PER):
                        rotary(banks[h], c_t, s_t,
                               kT[h][:, (tq - 1) * 512:tq * 512], f"k{tq-1}_{h}")

            # ---------------- V phase (transpose deferred one tile) ----------------
            wv_sb = load_w(wv_d, "v")
            pend_v = {}
            for tq in range(NQT + 1):
                if tq < NQT:
                    banks = proj_tile(xTv_d, wv_sb, tq, "v")
                    for h in range(H_PER):
                        vt = vtstp.tile([128, 512], f32r, tag="vt",
                                        name=f"vt_{tq}_{h}")
                        nc.scalar.copy(out=vt, in_=banks[h])
                        pend_v.setdefault(tq, []).append(vt)
                if tq - 1 in pend_v:
                    for h, vt in enumerate(pend_v.pop(tq - 1)):
                        tr = ps.tile([128, 512], f32, tag="bank",
                                     name=f"tr_{tq-1}_{h}")
                        for u in range(4):
                            nc.tensor.transpose(
                                out=tr[:, u * 128:(u + 1) * 128].bitcast(f32r),
                                in_=vt[:, u * 128:(u + 1) * 128],
                                identity=ident_sb).annotate("vtrans")
                        nc.scalar.copy(
                            out=v_sb[h][:, (tq - 1) * 512:tq * 512], in_=tr)

            # ------------- Q + attention + W_O, per q tile -------------
            # Per j: attention(j) with per-head deferred normalize chains,
            # then q-proj(j+1) + rotary(j+1), then W_O(j): the PE never sits
            # behind the recip/broadcast chain or the wo DMAs.
            wq_sb = load_w(wq_d, "q")

            def qproj_rot(j):
                banks = proj_tile(xTq_d, wq_sb, j, "q")
                c_t, s_t = load_cs(j, "q")
                tiles = []
                for h in range(H_PER):
                    qt = qttp.tile([128, 512], f32r, tag="qtt", name=f"qT_{j}_{h}")
                    rotary(banks[h], c_t, s_t, qt, f"q{j}_{h}")
                    tiles.append(qt)
                return tiles

            qTt = qproj_rot(0)
            for j in range(NQT):
                wo_tiles = {}
                for dd in range(2):
                    for h in range(H_PER):
                        wo_tiles[(dd, h)] = load_wo(h, dd, j)

                ztn_tiles = {}
                pending_fin = []

                def finalize(h, zt, den):
                    recip = smalls.tile([1, 512], f32r, tag="recip",
                                        name=f"recip_{j}_{h}")
                    with nc.allow_low_precision(reason="softmax denom in e8m11"):
                        nc.vector.reciprocal(out=recip, in_=den[0:1, :])
                    rb_ps = ps.tile([128, 512], f32, tag="bank", name=f"rb_{j}_{h}")
                    nc.tensor.matmul(out=rb_ps, lhsT=ones1_sb, rhs=recip,
                                     start=True, stop=True).annotate("rbMM")
                    rb_sb = rbp.tile([128, 512], f32, tag="rb", name=f"rbs_{j}_{h}")
                    nc.vector.tensor_copy(out=rb_sb, in_=rb_ps)
                    ztn = ztnp.tile([128, 512], f32r, tag="ztn", name=f"ztn_{j}_{h}")
                    nc.vector.tensor_mul(ztn, zt, rb_sb)
                    ztn_tiles[h] = ztn

                for h in range(H_PER):
                    n_k = 4 * j + 4
                    LOOK = 3
                    exps = {}
                    zt = ps.tile([128, 512], f32, tag="bank", name=f"zt_{j}_{h}")
                    den = ps.tile([2, 512], f32, tag="bank", name=f"den_{j}_{h}")
                    for ii in range(n_k + LOOK):
                        if ii < n_k:
                            i = ii
                            d = max(0, (i - 4 * j)) * 128
                            sc = ps.tile([128, 512], f32, tag="bank",
                                         name=f"sc_{j}_{h}_{i}")
                            nc.tensor.matmul(
                                out=sc[:, d:512],
                                lhsT=kT[h][:, i * 128:(i + 1) * 128],
                                rhs=qTt[h][:, d:512], start=True, stop=True
                                ).annotate("scoreMM")
                            ex = expp.tile([128, 512], f32r, tag="exp",
                                           name=f"ex_{j}_{h}_{i}")
                            nc.scalar.activation(out=ex[:, d:512], in_=sc[:, d:512],
                                                 func=AF.Exp)
                            if i >= 4 * j:
                                nc.vector.tensor_mul(
                                    ex[:, d:d + 128],
                                    ex[:, d:d + 128].bitcast(f32), triu_sb)
                            exps[i] = (ex, d)
                        if ii == 1 and pending_fin:
                            # previous head's normalize chain, now stall-free
                            finalize(*pending_fin.pop())
                        if ii >= LOOK:
                            i = ii - LOOK
                            ex, d = exps.pop(i)
                            nc.tensor.matmul(out=den[:, d:512], lhsT=ones2_sb,
                                             rhs=ex[:, d:512],
                                             start=(i == 0), stop=(i == n_k - 1)
                                             ).annotate("denMM")
                            nc.tensor.matmul(out=zt[:, d:512],
                                             lhsT=v_sb[h][:, i * 128:(i + 1) * 128],
                                             rhs=ex[:, d:512],
                                             start=(i == 0), stop=(i == n_k - 1)
                                             ).annotate("pvMM")
                    pending_fin.append((h, zt, den))
                    if h == 1:
                        for dd in range(2, 4):
                            for hh in range(H_PER):
                                wo_tiles[(dd, hh)] = load_wo(hh, dd, j)

                finalize(*pending_fin.pop())
                if j + 1 < NQT:
                    next_qTt = qproj_rot(j + 1)
                else:
                    next_qTt = None

                for dd in range(4):
                    for tt in range(4):
                        ops = ps.tile([128, 512], f32, tag="bank",
                                      name=f"o_{j}_{dd}_{tt}")
                        for h in range(H_PER):
                            nc.tensor.matmul(
                                out=ops,
                                lhsT=ztn_tiles[h][:, tt * 128:(tt + 1) * 128],
                                rhs=wo_tiles[(dd, h)],
                                start=(h == 0), stop=(h == H_PER - 1)
                                ).annotate("woMM")
                        osb = outstp.tile([128, 512], f32, tag="osb",
                                          name=f"osb_{j}_{dd}_{tt}")
                        nc.scalar.copy(out=osb, in_=ops)
                        nc.scalar.dma_start(
                            out=out_d.ap()[j * 512 + tt * 128:
                                           j * 512 + (tt + 1) * 128,
                                           dd * 512:(dd + 1) * 512],
                            in_=osb)
                qTt = next_qTt
    nc.compile()
    return nc


def _host_tables():
    pos = np.arange(S, dtype=np.float32)
    dim = np.arange(E // 2, dtype=np.float32)
    freq = (ROTARY_BASE ** (dim / (E / 2))).astype(np.float32)
    ang = pos[:, None] / freq[None, :]          # [S, 64]
    cosH = np.cos(ang).T.astype(np.float32)     # [64, S]
    sinH = np.sin(ang).T.astype(np.float32)
    cosT = np.concatenate([cosH, cosH], axis=0)             # [128, S]
    sinTs = np.concatenate([-sinH, sinH], axis=0)           # signed for swap-mul
    triu = np.triu(np.ones((128, 128), dtype=np.float32))   # valid: k_local <= q_local
    return cosT, sinTs, triu


def _numpy_fallback(query_input, key_input, value_input, W_Q, W_K, W_V, W_O,
                    b_Q, b_K, b_V, b_O):
    q = np.einsum("bpd,hde->bphe", query_input, W_Q) + b_Q
    k = np.einsum("bpd,hde->bphe", key_input, W_K) + b_K
    v = np.einsum("bpd,hde->bphe", value_input, W_V) + b_V
    cosT, sinTs, _ = _host_tables()
    cos = cosT.T[None, :, None, :]
    sin = np.concatenate([sinTs[64:], sinTs[64:]], axis=0).T[None, :, None, :]

    def rot(x):
        half = np.concatenate([-x[..., 64:], x[..., :64]], axis=-1)
        return x * cos + half * sin

    q, k = rot(q), rot(k)
    s = np.einsum("bqhe,bkhe->bhqk", q, k) / ATTN_SCALE
    mask = np.tril(np.ones((S, S), dtype=bool))
    s = np.where(mask[None, None], s, -np.inf)
    s = s - s.max(-1, keepdims=True)
    p = np.exp(s)
    p /= p.sum(-1, keepdims=True)
    z = np.einsum("bkhe,bhqk->bqhe", v, p)
    return (np.einsum("bqhe,hed->bqd", z, W_O) + b_O).astype(np.float32)


def _get_nc():
    if "nc" not in _CACHE:
        _CACHE["nc"] = _build_nc()
    return _CACHE["nc"]


def _make_in_maps(query_input, key_input, value_input, W_Q, W_K, W_V, W_O):
    cosT, sinTs, triu = _host_tables()
    consts = {
        "cosT": cosT, "sinTs": sinTs, "triu": triu,
        "ones2": np.ones((128, 2), np.float32),
        "ones1": np.ones((1, 128), np.float32),
        "ident": np.eye(128, dtype=np.float32),
    }
    xT = {}
    for b in range(B):
        xT[("q", b)] = _round_fp32r(query_input[b].T)
        xT[("k", b)] = _round_fp32r(key_input[b].T)
        xT[("v", b)] = _round_fp32r(value_input[b].T)
    wq_r = _round_fp32r(W_Q / ATTN_SCALE)
    wk_r = _round_fp32r(W_K)
    wv_r = _round_fp32r(W_V)
    wo_r = _round_fp32r(W_O)

    in_maps = []
    for c in range(N_CORES):
        b, hg = c // 4, c % 4
        h0 = hg * H_PER
        in_maps.append({
            "xTq": xT[("q", b)], "xTk": xT[("k", b)], "xTv": xT[("v", b)],
            "wq": wq_r[h0:h0 + H_PER], "wk": wk_r[h0:h0 + H_PER],
            "wv": wv_r[h0:h0 + H_PER], "wo": wo_r[h0:h0 + H_PER],
            **consts,
        })
    return in_maps


def kernel(query_input, key_input, value_input, W_Q, W_K, W_V, W_O,
           b_Q, b_K, b_V, b_O):
    if (np.abs(b_Q).max() > 0 or np.abs(b_K).max() > 0 or np.abs(b_V).max() > 0):
        # spec fills q/k/v biases with zeros; exact fallback just in case
        return _numpy_fallback(query_input, key_input, value_input,
                               W_Q, W_K, W_V, W_O, b_Q, b_K, b_V, b_O)

    from concourse import bass_utils

    nc = _get_nc()
    in_maps = _make_in_maps(query_input, key_input, value_input,
                            W_Q, W_K, W_V, W_O)
    res = bass_utils.run_bass_kernel_spmd(nc, in_maps,
                                          core_ids=list(range(N_CORES)))
    out = np.zeros((B, S, DM), dtype=np.float32)
    for c in range(N_CORES):
        out[c // 4] += res.results[c]["out"]
    out += b_O.astype(np.float32)[None, None, :]
    return out
